# revision 1
# baseline (speedup 1.0000x reference)
"""AbilityEncoder TRN2 kernel v2.

psA = tblA42 @ encA   (42 rows: tag26+op15+b2cnt; leaf cols all-zero ->
relu(0)=0), single matmul at rows 0-41.
psB = tblB @ encL     (26 rows; nonleaf cols zero), single matmuls on
row-strips 64-89 (even lvl2 nodes) / 96-121 (odd).
lvl1: pre1_j = W2W1@sh_j + W1bot@sl_j + tblA42@encA(1+j), one accumulation
group, all at tile position (0,0). Root analogous + b2c bias. Output via
h0-stationary matmul (tree-major).
fp8 one-hot encodings (exact) x bf16 tables; relu evac split ACT/DVE.
"""
import numpy as np
import ml_dtypes
import sys

sys.path.insert(0, "/opt/trn_rl_repo")

H = 96
NODES = 21
N = 32768
NCORES = 8
NPC = N // NCORES
T = 512
NT = NPC // T
RA = 42   # encA rows
RL = 26   # encL rows
F8 = ml_dtypes.float8_e4m3
BF = ml_dtypes.bfloat16
# lvl2 pair-relu engine pattern: A=scalar, D=vector, interleaved
RELU_PAT = "ADAADADAADADAADA"  # 10 A, 6 D


def _build_host_tensors(trigger_ids, action_ids, target_ids, operand_ids,
                        trig_table, eff_table, targ_table, op_table,
                        W1, b1, W2, b2, Wl, bl):
    f64 = np.float64
    W1 = W1.astype(f64); W2 = W2.astype(f64); Wl = Wl.astype(f64)
    W1top, W1bot = W1[:H], W1[H:]
    W2W1 = W2 @ W1bot

    tblA = np.zeros((RA, H), f64)
    tblA[0:7] = trig_table.astype(f64) @ W1bot + b1.astype(f64)
    tblA[7:16] = eff_table.astype(f64) @ W1bot
    tblA[16:26] = targ_table.astype(f64) @ W1bot
    tblA[26:41] = op_table.astype(f64) @ W1top
    tblA[41] = b2.astype(f64) @ W1bot
    tblB = np.zeros((RL, H), f64)
    tblB[0:7] = trig_table.astype(f64) @ Wl + bl.astype(f64)
    tblB[7:16] = eff_table.astype(f64) @ Wl
    tblB[16:26] = targ_table.astype(f64) @ Wl

    tbls = np.zeros((128, H), f64)
    tbls[0:RA] = tblA
    tbls[64:64 + RL] = tblB
    tbls[96:96 + RL] = tblB

    W2aug = np.zeros((H + 1, H), f64)
    W2aug[:H] = W2
    W2aug[H] = b2.astype(f64)
    b2w1x4 = 4.0 * (b2.astype(f64) @ W1bot)

    t = trigger_ids.astype(np.int64); a = action_ids.astype(np.int64)
    g = target_ids.astype(np.int64); o = operand_ids.astype(np.int64)
    leaf = (o == 0)
    cols = np.arange(N)
    hasb2 = bool(np.any(b2 != 0))

    encA = np.zeros((RA, NODES, N), F8)
    encLe = np.zeros((RL, 8, N), F8)
    encLo = np.zeros((RL, 8, N), F8)
    for n in range(NODES):
        if n < 5:
            encA[:, n][t[:, n], cols] = 1.0
            encA[:, n][7 + a[:, n], cols] = 1.0
            encA[:, n][16 + g[:, n], cols] = 1.0
            encA[:, n][26 + o[:, n], cols] = 1.0
            if hasb2 and n >= 1:
                j = n - 1
                cnt = (~leaf[:, 5 + 4 * j:5 + 4 * j + 4]).sum(axis=1)
                encA[:, n][41, cols] = cnt.astype(np.float64)
        else:
            lf = leaf[:, n]
            nl = ~lf
            encA[:, n][t[nl, n], cols[nl]] = 1.0
            encA[:, n][7 + a[nl, n], cols[nl]] = 1.0
            encA[:, n][16 + g[nl, n], cols[nl]] = 1.0
            encA[:, n][26 + o[nl, n], cols[nl]] = 1.0
            p = n - 5
            dst = encLe if p % 2 == 0 else encLo
            k = p // 2
            dst[:, k][t[lf, n], cols[lf]] = 1.0
            dst[:, k][7 + a[lf, n], cols[lf]] = 1.0
            dst[:, k][16 + g[lf, n], cols[lf]] = 1.0

    return (encA, encLe, encLo, tbls.astype(BF),
            W2W1.astype(BF), W1bot.astype(BF), W2aug.astype(BF),
            b2w1x4.astype(np.float32).reshape(H, 1))


_CACHED = {}
_SKIP_SPLIT = False


def _patch_tile(tile, mybir, _br, ScopedClock):
    def _drain_and_barrier(self, tick_clock, wait_clock):
        nc_ = self.nc
        probe = nc_.sync.drain()
        wait_clock.add_sem_waits(probe.ins,
                                 ScopedClock({None: tick_clock.global_clock}))
        si = probe.ins.sync_info
        waits = list(si.on_wait) if si is not None else []
        if len(waits) > 1:
            si.on_wait = waits[:1]
            for w in waits[1:]:
                extra = nc_.sync.drain()
                extra.ins.sync_info = _br.SyncInfo(on_wait=[w], on_update=[])
        nc_.all_engine_barrier()
        popped = nc_._tile_sem_poison_stack.pop()
        assert popped is self._sem_poison
        nc_.clear_and_free_semaphores(list(self.sems.allocated().values()))
        nc_.all_engine_barrier()

    tile.TileContext._drain_and_barrier = _drain_and_barrier


def _split_waits(nc_, mybir, _br, max_waits=1):
    for f in nc_.m.functions:
        for bb in f.blocks:
            out = []
            for inst in bb.instructions:
                si = inst.sync_info
                if si is not None:
                    waits = list(si.on_wait)
                    if len(waits) > max_waits:
                        extra, keep = waits[:-max_waits], waits[-max_waits:]
                        for j, w in enumerate(extra):
                            ev = mybir.InstEventSemaphore(
                                name=f"{inst.name}-xw{j}")
                            ev.engine = inst.engine
                            ev.sync_info = _br.SyncInfo(
                                on_wait=[w], on_update=[])
                            out.append(ev)
                        si.on_wait = keep
                out.append(inst)
            try:
                bb.instructions = out
            except Exception:
                bb.instructions.clear()
                for i_ in out:
                    bb.instructions.append(i_)


def _build_program():
    import concourse.bass as bass
    import concourse.tile as tile
    import concourse.mybir as mybir
    from concourse.vector_clock import ScopedClock
    import bass_rust as _br

    _patch_tile(tile, mybir, _br, ScopedClock)

    dt = mybir.dt
    Relu = mybir.ActivationFunctionType.Relu
    ADD = mybir.AluOpType.add
    nc = bass.Bass(trn_type="TRN2", target_bir_lowering=False, debug=False)
    encA_d = nc.dram_tensor("encA", [RA, NT * NODES * T], dt.float8e4,
                            kind="ExternalInput").ap()
    encLe_d = nc.dram_tensor("encLe", [RL, NT * 8 * T], dt.float8e4,
                             kind="ExternalInput").ap()
    encLo_d = nc.dram_tensor("encLo", [RL, NT * 8 * T], dt.float8e4,
                             kind="ExternalInput").ap()
    tbls_d = nc.dram_tensor("tbls", [128, H], dt.bfloat16,
                            kind="ExternalInput").ap()
    w2w1_d = nc.dram_tensor("w2w1", [H, H], dt.bfloat16,
                            kind="ExternalInput").ap()
    w1bot_d = nc.dram_tensor("w1bot", [H, H], dt.bfloat16,
                             kind="ExternalInput").ap()
    w2aug_d = nc.dram_tensor("w2aug", [H + 1, H], dt.bfloat16,
                             kind="ExternalInput").ap()
    b2c_d = nc.dram_tensor("b2c", [H, 1], dt.float32,
                           kind="ExternalInput").ap()
    out_d = nc.dram_tensor("out", [NPC, H], dt.float32,
                           kind="ExternalOutput").ap()

    with tile.TileContext(nc) as tc:
        with tc.tile_pool(name="const", bufs=1) as cpool, \
             tc.tile_pool(name="enc", bufs=3) as epool, \
             tc.tile_pool(name="hbuf", bufs=3) as hpool, \
             tc.tile_pool(name="ps", bufs=3, space="PSUM") as pspool, \
             tc.tile_pool(name="pst", bufs=1, space="PSUM") as ptpool:

            tbls_s = cpool.tile([128, H], dt.bfloat16)
            nc.sync.dma_start(tbls_s[:], tbls_d[:])
            w2w1_s = cpool.tile([H, H], dt.bfloat16)
            nc.sync.dma_start(w2w1_s[:], w2w1_d[:])
            w1bot_s = cpool.tile([H, H], dt.bfloat16)
            nc.sync.dma_start(w1bot_s[:], w1bot_d[:])
            w2aug_s = cpool.tile([H + 1, H], dt.bfloat16)
            nc.sync.dma_start(w2aug_s[:], w2aug_d[:])
            b2c_s = cpool.tile([H, 1], dt.float32)
            nc.sync.dma_start(b2c_s[:], b2c_d[:])

            tbA = tbls_s[0:RA, :]
            tbBe = tbls_s[64:64 + RL, :]
            tbBo = tbls_s[96:96 + RL, :]

            for it in range(NT):
                enc = epool.tile([128, NODES * T], dt.float8e4, tag="enc")
                nc.sync.dma_start(
                    enc[0:RA, :],
                    encA_d[:, it * NODES * T:(it + 1) * NODES * T])
                nc.sync.dma_start(
                    enc[64:64 + RL, 0:8 * T],
                    encLe_d[:, it * 8 * T:(it + 1) * 8 * T])
                nc.sync.dma_start(
                    enc[96:96 + RL, 0:8 * T],
                    encLo_d[:, it * 8 * T:(it + 1) * 8 * T])

                def eA(n):
                    return enc[0:RA, n * T:(n + 1) * T]

                def eL(p):
                    k = p // 2
                    if p % 2 == 0:
                        return enc[64:64 + RL, k * T:(k + 1) * T]
                    return enc[96:96 + RL, k * T:(k + 1) * T]

                hA = []
                hB = []
                nrelu = 0
                for j in range(4):
                    hA_j = hpool.tile([H, 4, T], dt.bfloat16, tag="hA")
                    hB_j = hpool.tile([H, 4, T], dt.bfloat16, tag="hB")
                    hA.append(hA_j)
                    hB.append(hB_j)
                    for k in range(2):
                        psA = pspool.tile([128, 2 * T], dt.float32, tag="ps")
                        psB = pspool.tile([128, 2 * T], dt.float32, tag="ps")
                        for c in (2 * k, 2 * k + 1):
                            n = 5 + 4 * j + c
                            p = n - 5
                            o_ = (c - 2 * k) * T
                            nc.tensor.matmul(psA[0:H, o_:o_ + T], tbA, eA(n),
                                             start=True, stop=True)
                            if p % 2 == 0:
                                nc.tensor.matmul(psB[0:H, o_:o_ + T], tbBe,
                                                 eL(p), start=True, stop=True)
                            else:
                                nc.tensor.matmul(psB[0:H, o_:o_ + T], tbBo,
                                                 eL(p), start=True, stop=True,
                                                 tile_position=(96, 0))
                        if RELU_PAT[nrelu] == "A":
                            nc.scalar.activation(hA_j[:, 2 * k:2 * k + 2, :],
                                                 psA[0:H, :], Relu)
                        else:
                            nc.vector.tensor_scalar_max(
                                hA_j[:, 2 * k:2 * k + 2, :], psA[0:H, :], 0.0)
                        nrelu += 1
                        if RELU_PAT[nrelu] == "A":
                            nc.scalar.activation(hB_j[:, 2 * k:2 * k + 2, :],
                                                 psB[0:H, :], Relu)
                        else:
                            nc.vector.tensor_scalar_max(
                                hB_j[:, 2 * k:2 * k + 2, :], psB[0:H, :], 0.0)
                        nrelu += 1

                # sibling sums
                sh = []
                sl = []
                for j in range(4):
                    sh_j = hpool.tile([H, T], dt.bfloat16, tag="sh")
                    sl_j = hpool.tile([H, T], dt.bfloat16, tag="sl")
                    sh.append(sh_j)
                    sl.append(sl_j)
                    tmpA = hpool.tile([H, 2, T], dt.bfloat16, tag="tmpA")
                    nc.vector.tensor_tensor(out=tmpA[:], in0=hA[j][:, 0:2, :],
                                            in1=hA[j][:, 2:4, :], op=ADD)
                    nc.vector.tensor_tensor(out=sh_j[:], in0=tmpA[:, 0, :],
                                            in1=tmpA[:, 1, :], op=ADD)
                    tmpB = hpool.tile([H, 2, T], dt.bfloat16, tag="tmpB")
                    nc.vector.tensor_tensor(out=tmpB[:], in0=hB[j][:, 0:2, :],
                                            in1=hB[j][:, 2:4, :], op=ADD)
                    nc.vector.tensor_tensor(out=sl_j[:], in0=tmpB[:, 0, :],
                                            in1=tmpB[:, 1, :], op=ADD)

                # level 1
                h1 = hpool.tile([H, 4, T], dt.bfloat16, tag="h1")
                for half in range(2):
                    ps1 = ptpool.tile([128, 2 * T], dt.float32, tag="pst")
                    for q in range(2):
                        j = 2 * half + q
                        o_ = q * T
                        dst = ps1[0:H, o_:o_ + T]
                        nc.tensor.matmul(dst, w2w1_s[:], sh[j][:],
                                         start=True, stop=False)
                        nc.tensor.matmul(dst, w1bot_s[:], sl[j][:],
                                         start=False, stop=False)
                        nc.tensor.matmul(dst, tbA, eA(1 + j),
                                         start=False, stop=True)
                    nc.scalar.activation(h1[:, 2 * half:2 * half + 2, :],
                                         ps1[0:H, :], Relu)

                # root
                s0 = hpool.tile([H, T], dt.bfloat16, tag="s0")
                tmp0 = hpool.tile([H, 2, T], dt.bfloat16, tag="tmp0")
                nc.vector.tensor_tensor(out=tmp0[:], in0=h1[:, 0:2, :],
                                        in1=h1[:, 2:4, :], op=ADD)
                nc.vector.tensor_tensor(out=s0[:], in0=tmp0[:, 0, :],
                                        in1=tmp0[:, 1, :], op=ADD)
                ps0 = ptpool.tile([128, 2 * T], dt.float32, tag="pst")
                nc.tensor.matmul(ps0[0:H, 0:T], w2w1_s[:], s0[:],
                                 start=True, stop=False)
                nc.tensor.matmul(ps0[0:H, 0:T], tbA, eA(0),
                                 start=False, stop=True)
                h0 = hpool.tile([H + 1, T], dt.bfloat16, tag="h0")
                nc.scalar.activation(h0[0:H, :], ps0[0:H, 0:T], Relu,
                                     bias=b2c_s[:])
                nc.vector.memset(h0[H:H + 1, :], 1.0)

                # out = h0.T @ W2aug (tree-major)
                osb = hpool.tile([128, 4 * H], dt.float32, tag="osb")
                for pair in range(2):
                    pso = ptpool.tile([128, 2 * T], dt.float32, tag="pst")
                    for q in range(2):
                        ch = 2 * pair + q
                        off = q * T
                        nc.tensor.matmul(pso[:, off:off + H],
                                         h0[:, ch * 128:(ch + 1) * 128],
                                         w2aug_s[:], start=True, stop=True)
                    for q in range(2):
                        ch = 2 * pair + q
                        off = q * T
                        if ch % 2 == 0:
                            nc.vector.tensor_copy(
                                out=osb[:, ch * H:(ch + 1) * H],
                                in_=pso[:, off:off + H])
                        else:
                            nc.scalar.copy(osb[:, ch * H:(ch + 1) * H],
                                           pso[:, off:off + H])
                for ch in range(4):
                    nc.scalar.dma_start(
                        out_d[it * T + ch * 128:it * T + (ch + 1) * 128, :],
                        osb[:, ch * H:(ch + 1) * H])

    if not _SKIP_SPLIT:
        _split_waits(nc, mybir, _br)
    return nc


def _make_in_maps(host):
    (encA, encLe, encLo, tbls, W2W1, W1bot, W2aug, b2c) = host
    in_maps = []
    for cix in range(NCORES):
        s = slice(cix * NPC, (cix + 1) * NPC)

        def tilefmt(e, nn):
            a = e[:, :, s].reshape(e.shape[0], nn, NT, T).transpose(0, 2, 1, 3)
            return np.ascontiguousarray(a.reshape(e.shape[0], NT * nn * T))

        in_maps.append({
            "encA": tilefmt(encA, NODES),
            "encLe": tilefmt(encLe, 8),
            "encLo": tilefmt(encLo, 8),
            "tbls": tbls, "w2w1": W2W1, "w1bot": W1bot,
            "w2aug": W2aug, "b2c": b2c,
        })
    return in_maps


def kernel(**inputs) -> np.ndarray:
    from concourse.bass_utils import run_bass_kernel_spmd

    host = _build_host_tensors(**inputs)
    if "nc" not in _CACHED:
        _CACHED["nc"] = _build_program()
    nc = _CACHED["nc"]
    in_maps = _make_in_maps(host)
    res = run_bass_kernel_spmd(nc, in_maps, list(range(NCORES)))
    out = np.concatenate([res.results[c]["out"] for c in range(NCORES)],
                         axis=0)
    return out.astype(np.float32)



# revision 4
# speedup vs baseline: 1.1373x; 1.1373x over previous
"""AbilityEncoder TRN2 kernel v3.

Same algebra as v2 (one-hot fp8 encodings x host-folded tables):
  psA = tblA42 @ encA; psB = tblB @ encL; lvl1 via W2W1/W1bot accumulation;
  root analogous; output via h0-stationary transposed matmul.
v3 changes:
  - one-hot matmuls use fp8 DoubleRow with hi/lo-split tables (2x stream
    rate, ~0.4% table quantization error vs bf16)
  - PE warmup burst before the tile loop (HAM clock-gate latch)
  - output DMAs issued from the gpsimd queue (frees the scalar engine)
  - tunable relu-evacuation engine split
"""
import numpy as np
import ml_dtypes
import sys

sys.path.insert(0, "/opt/trn_rl_repo")

H = 96
NODES = 21
N = 32768
NCORES = 8
NPC = N // NCORES
T = 512
NT = NPC // T
RA = 42   # encA rows
RL = 26   # encL rows
F8 = ml_dtypes.float8_e4m3
BF = ml_dtypes.bfloat16

USE_DR = True        # DoubleRow fp8 hi/lo for one-hot matmuls
DR_BROADCAST = True  # broadcast rhs dup-dim (False: duplicated upload)
WARMUP_MMS = 0       # PE warmup matmuls before tile loop
SCALE = 4096.0       # table scale: lifts fp8 values out of denormal range
# lvl2 pair-relu engine pattern: A=scalar(ACT), D=vector(DVE)
RELU_PAT = "ADAADADAADADAADA"  # 10 A, 6 D


def _hl_split(t):
    hi = t.astype(F8)
    lo = (t - hi.astype(np.float64)).astype(F8)
    return np.stack([hi, lo], axis=1)  # [rows, 2, H]


def _build_host_tensors(trigger_ids, action_ids, target_ids, operand_ids,
                        trig_table, eff_table, targ_table, op_table,
                        W1, b1, W2, b2, Wl, bl):
    f64 = np.float64
    W1 = W1.astype(f64); W2 = W2.astype(f64); Wl = Wl.astype(f64)
    W1top, W1bot = W1[:H], W1[H:]
    W2W1 = W2 @ W1bot

    tblA = np.zeros((RA, H), f64)
    tblA[0:7] = trig_table.astype(f64) @ W1bot + b1.astype(f64)
    tblA[7:16] = eff_table.astype(f64) @ W1bot
    tblA[16:26] = targ_table.astype(f64) @ W1bot
    tblA[26:41] = op_table.astype(f64) @ W1top
    tblA[41] = b2.astype(f64) @ W1bot
    tblB = np.zeros((RL, H), f64)
    tblB[0:7] = trig_table.astype(f64) @ Wl + bl.astype(f64)
    tblB[7:16] = eff_table.astype(f64) @ Wl
    tblB[16:26] = targ_table.astype(f64) @ Wl

    # power-of-2 scale lifting fp8 table values out of the denormal range
    mx = max(np.abs(tblA).max(), np.abs(tblB).max(), 1e-30)
    S = SCALE
    while mx * S > 224.0:
        S /= 2.0
    while mx * S * 2.0 <= 224.0 and S < 65536.0:
        S *= 2.0
    tblA = tblA * S
    tblB = tblB * S
    W2W1 = W2W1 * S
    W1bot_s = W1bot * S

    # bf16 tables (fallback / reference)
    tbls = np.zeros((128, H), f64)
    tbls[0:RA] = tblA
    tbls[64:64 + RL] = tblB
    tbls[96:96 + RL] = tblB
    # fp8 hi/lo tables, [128, 2*H]
    tbls_hl = np.zeros((128, 2, H), F8)
    tbls_hl[0:RA] = _hl_split(tblA)
    tbls_hl[64:64 + RL] = _hl_split(tblB)
    tbls_hl[96:96 + RL] = _hl_split(tblB)

    W2aug = np.zeros((H + 1, H), f64)
    W2aug[:H] = W2
    W2aug[H] = b2.astype(f64)
    b2w1x4 = 4.0 * (b2.astype(f64) @ W1bot)

    t = trigger_ids.astype(np.int64); a = action_ids.astype(np.int64)
    g = target_ids.astype(np.int64); o = operand_ids.astype(np.int64)
    leaf = (o == 0)
    cols = np.arange(N)
    hasb2 = bool(np.any(b2 != 0))

    encA = np.zeros((RA, NODES, N), F8)
    encLe = np.zeros((RL, 8, N), F8)
    encLo = np.zeros((RL, 8, N), F8)
    for n in range(NODES):
        if n < 5:
            encA[:, n][t[:, n], cols] = 1.0
            encA[:, n][7 + a[:, n], cols] = 1.0
            encA[:, n][16 + g[:, n], cols] = 1.0
            encA[:, n][26 + o[:, n], cols] = 1.0
            if hasb2 and n >= 1:
                j = n - 1
                cnt = (~leaf[:, 5 + 4 * j:5 + 4 * j + 4]).sum(axis=1)
                encA[:, n][41, cols] = cnt.astype(np.float64)
        else:
            lf = leaf[:, n]
            nl = ~lf
            encA[:, n][t[nl, n], cols[nl]] = 1.0
            encA[:, n][7 + a[nl, n], cols[nl]] = 1.0
            encA[:, n][16 + g[nl, n], cols[nl]] = 1.0
            encA[:, n][26 + o[nl, n], cols[nl]] = 1.0
            p = n - 5
            dst = encLe if p % 2 == 0 else encLo
            k = p // 2
            dst[:, k][t[lf, n], cols[lf]] = 1.0
            dst[:, k][7 + a[lf, n], cols[lf]] = 1.0
            dst[:, k][16 + g[lf, n], cols[lf]] = 1.0

    return (encA, encLe, encLo, tbls.astype(BF), tbls_hl.reshape(128, 2 * H),
            W2W1.astype(BF), W1bot_s.astype(BF), W2aug.astype(BF),
            b2w1x4.astype(np.float32).reshape(H, 1), S)


_CACHED = {}
_SKIP_SPLIT = False


def _patch_tile(tile, mybir, _br, ScopedClock):
    def _drain_and_barrier(self, tick_clock, wait_clock):
        nc_ = self.nc
        probe = nc_.sync.drain()
        wait_clock.add_sem_waits(probe.ins,
                                 ScopedClock({None: tick_clock.global_clock}))
        si = probe.ins.sync_info
        waits = list(si.on_wait) if si is not None else []
        if len(waits) > 1:
            si.on_wait = waits[:1]
            for w in waits[1:]:
                extra = nc_.sync.drain()
                extra.ins.sync_info = _br.SyncInfo(on_wait=[w], on_update=[])
        nc_.all_engine_barrier()
        popped = nc_._tile_sem_poison_stack.pop()
        assert popped is self._sem_poison
        nc_.clear_and_free_semaphores(list(self.sems.allocated().values()))
        nc_.all_engine_barrier()

    tile.TileContext._drain_and_barrier = _drain_and_barrier


def _split_waits(nc_, mybir, _br, max_waits=1):
    for f in nc_.m.functions:
        for bb in f.blocks:
            out = []
            for inst in bb.instructions:
                si = inst.sync_info
                if si is not None:
                    waits = list(si.on_wait)
                    if len(waits) > max_waits:
                        extra, keep = waits[:-max_waits], waits[-max_waits:]
                        for j, w in enumerate(extra):
                            ev = mybir.InstEventSemaphore(
                                name=f"{inst.name}-xw{j}")
                            ev.engine = inst.engine
                            ev.sync_info = _br.SyncInfo(
                                on_wait=[w], on_update=[])
                            out.append(ev)
                        si.on_wait = keep
                out.append(inst)
            try:
                bb.instructions = out
            except Exception:
                bb.instructions.clear()
                for i_ in out:
                    bb.instructions.append(i_)


def _build_program(S):
    import concourse.bass as bass
    import concourse.tile as tile
    import concourse.mybir as mybir
    from concourse.vector_clock import ScopedClock
    import bass_rust as _br

    _patch_tile(tile, mybir, _br, ScopedClock)

    dt = mybir.dt
    Relu = mybir.ActivationFunctionType.Relu
    ADD = mybir.AluOpType.add
    MAXOP = mybir.AluOpType.max
    MULT = mybir.AluOpType.mult
    INV = 1.0 / S
    DRmode = mybir.MatmulPerfMode.DoubleRow if USE_DR else None
    ETB = 2 * T if (USE_DR and not DR_BROADCAST) else T  # enc bytes per node

    nc = bass.Bass(trn_type="TRN2", target_bir_lowering=False, debug=False)
    encA_d = nc.dram_tensor("encA", [RA, NT * NODES * ETB], dt.float8e4,
                            kind="ExternalInput").ap()
    encLe_d = nc.dram_tensor("encLe", [RL, NT * 8 * ETB], dt.float8e4,
                             kind="ExternalInput").ap()
    encLo_d = nc.dram_tensor("encLo", [RL, NT * 8 * ETB], dt.float8e4,
                             kind="ExternalInput").ap()
    tbls_d = nc.dram_tensor("tbls", [128, H], dt.bfloat16,
                            kind="ExternalInput").ap()
    tblshl_d = nc.dram_tensor("tblshl", [128, 2 * H], dt.float8e4,
                              kind="ExternalInput").ap()
    w2w1_d = nc.dram_tensor("w2w1", [H, H], dt.bfloat16,
                            kind="ExternalInput").ap()
    w1bot_d = nc.dram_tensor("w1bot", [H, H], dt.bfloat16,
                             kind="ExternalInput").ap()
    w2aug_d = nc.dram_tensor("w2aug", [H + 1, H], dt.bfloat16,
                             kind="ExternalInput").ap()
    b2c_d = nc.dram_tensor("b2c", [H, 1], dt.float32,
                           kind="ExternalInput").ap()
    out_d = nc.dram_tensor("out", [NPC, H], dt.float32,
                           kind="ExternalOutput").ap()

    with tile.TileContext(nc) as tc:
        with tc.tile_pool(name="const", bufs=1) as cpool, \
             tc.tile_pool(name="enc", bufs=3) as epool, \
             tc.tile_pool(name="hbuf", bufs=3) as hpool, \
             tc.tile_pool(name="ps", bufs=4, space="PSUM") as pspool:

            tbls_s = cpool.tile([128, H], dt.bfloat16)
            nc.sync.dma_start(tbls_s[:], tbls_d[:])
            tblshl_s = cpool.tile([128, 2 * H], dt.float8e4)
            nc.sync.dma_start(tblshl_s[:], tblshl_d[:])
            w2w1_s = cpool.tile([H, H], dt.bfloat16)
            nc.sync.dma_start(w2w1_s[:], w2w1_d[:])
            w1bot_s = cpool.tile([H, H], dt.bfloat16)
            nc.sync.dma_start(w1bot_s[:], w1bot_d[:])
            w2aug_s = cpool.tile([H + 1, H], dt.bfloat16)
            nc.sync.dma_start(w2aug_s[:], w2aug_d[:])
            b2c_s = cpool.tile([H, 1], dt.float32)
            nc.sync.dma_start(b2c_s[:], b2c_d[:])

            if USE_DR:
                tbA = tblshl_s[0:RA, :].rearrange("p (a b) -> p a b", a=2)
                tbBe = tblshl_s[64:64 + RL, :].rearrange(
                    "p (a b) -> p a b", a=2)
                tbBo = tblshl_s[96:96 + RL, :].rearrange(
                    "p (a b) -> p a b", a=2)
            else:
                tbA = tbls_s[0:RA, :]
                tbBe = tbls_s[64:64 + RL, :]
                tbBo = tbls_s[96:96 + RL, :]

            # PE warmup burst: latch the HAM clock gate to full rate while
            # the first enc tile DMA streams in.
            if WARMUP_MMS:
                warm = cpool.tile([H, T], dt.bfloat16)
                nc.vector.memset(warm[:], 0.0)
                psw = pspool.tile([128, 2 * T], dt.float32, tag="ps")
                for w in range(WARMUP_MMS):
                    nc.tensor.matmul(psw[0:H, 0:T], w2w1_s[:], warm[:],
                                     start=True, stop=True,
                                     skip_group_check=True)

            for it in range(NT):
                enc = epool.tile([128, NODES * ETB], dt.float8e4, tag="enc")
                nc.sync.dma_start(
                    enc[0:RA, :],
                    encA_d[:, it * NODES * ETB:(it + 1) * NODES * ETB])
                nc.sync.dma_start(
                    enc[64:64 + RL, 0:8 * ETB],
                    encLe_d[:, it * 8 * ETB:(it + 1) * 8 * ETB])
                nc.sync.dma_start(
                    enc[96:96 + RL, 0:8 * ETB],
                    encLo_d[:, it * 8 * ETB:(it + 1) * 8 * ETB])

                def eA(n):
                    blk = enc[0:RA, n * ETB:(n + 1) * ETB]
                    if not USE_DR:
                        return blk
                    if DR_BROADCAST:
                        return blk.unsqueeze(1).broadcast_to([RA, 2, T])
                    return blk.rearrange("p (a b) -> p a b", a=2)

                def eL(p):
                    k = p // 2
                    row0 = 64 if p % 2 == 0 else 96
                    blk = enc[row0:row0 + RL, k * ETB:(k + 1) * ETB]
                    if not USE_DR:
                        return blk
                    if DR_BROADCAST:
                        return blk.unsqueeze(1).broadcast_to([RL, 2, T])
                    return blk.rearrange("p (a b) -> p a b", a=2)

                hA = []
                hB = []
                nrelu = 0
                for j in range(4):
                    hA_j = hpool.tile([H, 4, T], dt.bfloat16, tag="hA")
                    hB_j = hpool.tile([H, 4, T], dt.bfloat16, tag="hB")
                    hA.append(hA_j)
                    hB.append(hB_j)
                    for k in range(2):
                        psA = pspool.tile([128, 2 * T], dt.float32, tag="ps")
                        psB = pspool.tile([128, 2 * T], dt.float32, tag="ps")
                        for c in (2 * k, 2 * k + 1):
                            n = 5 + 4 * j + c
                            p = n - 5
                            o_ = (c - 2 * k) * T
                            nc.tensor.matmul(psA[0:H, o_:o_ + T], tbA, eA(n),
                                             start=True, stop=True,
                                             perf_mode=DRmode)
                            if p % 2 == 0:
                                nc.tensor.matmul(psB[0:H, o_:o_ + T], tbBe,
                                                 eL(p), start=True, stop=True,
                                                 perf_mode=DRmode,
                                                 tile_position=(64, 0))
                            else:
                                nc.tensor.matmul(psB[0:H, o_:o_ + T], tbBo,
                                                 eL(p), start=True, stop=True,
                                                 perf_mode=DRmode,
                                                 tile_position=(96, 0))
                        if RELU_PAT[nrelu] == "A":
                            nc.scalar.activation(hA_j[:, 2 * k:2 * k + 2, :],
                                                 psA[0:H, :], Relu, scale=INV)
                        else:
                            nc.vector.tensor_scalar(
                                hA_j[:, 2 * k:2 * k + 2, :], psA[0:H, :],
                                0.0, INV, op0=MAXOP, op1=MULT)
                        nrelu += 1
                        if RELU_PAT[nrelu] == "A":
                            nc.scalar.activation(hB_j[:, 2 * k:2 * k + 2, :],
                                                 psB[0:H, :], Relu, scale=INV)
                        else:
                            nc.vector.tensor_scalar(
                                hB_j[:, 2 * k:2 * k + 2, :], psB[0:H, :],
                                0.0, INV, op0=MAXOP, op1=MULT)
                        nrelu += 1

                # sibling sums
                sh = []
                sl = []
                for j in range(4):
                    sh_j = hpool.tile([H, T], dt.bfloat16, tag="sh")
                    sl_j = hpool.tile([H, T], dt.bfloat16, tag="sl")
                    sh.append(sh_j)
                    sl.append(sl_j)
                    tmpA = hpool.tile([H, 2, T], dt.bfloat16, tag="tmpA")
                    nc.vector.tensor_tensor(out=tmpA[:], in0=hA[j][:, 0:2, :],
                                            in1=hA[j][:, 2:4, :], op=ADD)
                    nc.vector.tensor_tensor(out=sh_j[:], in0=tmpA[:, 0, :],
                                            in1=tmpA[:, 1, :], op=ADD)
                    tmpB = hpool.tile([H, 2, T], dt.bfloat16, tag="tmpB")
                    nc.vector.tensor_tensor(out=tmpB[:], in0=hB[j][:, 0:2, :],
                                            in1=hB[j][:, 2:4, :], op=ADD)
                    nc.vector.tensor_tensor(out=sl_j[:], in0=tmpB[:, 0, :],
                                            in1=tmpB[:, 1, :], op=ADD)

                # level 1
                h1 = hpool.tile([H, 4, T], dt.bfloat16, tag="h1")
                for half in range(2):
                    ps1 = pspool.tile([128, 2 * T], dt.float32, tag="ps")
                    for q in range(2):
                        j = 2 * half + q
                        o_ = q * T
                        dst = ps1[0:H, o_:o_ + T]
                        nc.tensor.matmul(dst, tbA, eA(1 + j),
                                         start=True, stop=False,
                                         perf_mode=DRmode)
                        nc.tensor.matmul(dst, w2w1_s[:], sh[j][:],
                                         start=False, stop=False)
                        nc.tensor.matmul(dst, w1bot_s[:], sl[j][:],
                                         start=False, stop=True)
                    nc.scalar.activation(h1[:, 2 * half:2 * half + 2, :],
                                         ps1[0:H, :], Relu, scale=INV)

                # root
                s0 = hpool.tile([H, T], dt.bfloat16, tag="s0")
                tmp0 = hpool.tile([H, 2, T], dt.bfloat16, tag="tmp0")
                nc.vector.tensor_tensor(out=tmp0[:], in0=h1[:, 0:2, :],
                                        in1=h1[:, 2:4, :], op=ADD)
                nc.vector.tensor_tensor(out=s0[:], in0=tmp0[:, 0, :],
                                        in1=tmp0[:, 1, :], op=ADD)
                ps0 = pspool.tile([128, 2 * T], dt.float32, tag="ps")
                nc.tensor.matmul(ps0[0:H, 0:T], tbA, eA(0),
                                 start=True, stop=False, perf_mode=DRmode)
                nc.tensor.matmul(ps0[0:H, 0:T], w2w1_s[:], s0[:],
                                 start=False, stop=True)
                h0 = hpool.tile([H + 1, T], dt.bfloat16, tag="h0")
                nc.scalar.activation(h0[0:H, :], ps0[0:H, 0:T], Relu,
                                     bias=b2c_s[:], scale=INV)
                nc.vector.memset(h0[H:H + 1, :], 1.0)

                # out = h0.T @ W2aug (tree-major)
                osb = hpool.tile([128, 4 * H], dt.float32, tag="osb")
                for pair in range(2):
                    pso = pspool.tile([128, 2 * T], dt.float32, tag="ps")
                    for q in range(2):
                        ch = 2 * pair + q
                        off = q * T
                        nc.tensor.matmul(pso[:, off:off + H],
                                         h0[:, ch * 128:(ch + 1) * 128],
                                         w2aug_s[:], start=True, stop=True)
                    for q in range(2):
                        ch = 2 * pair + q
                        off = q * T
                        if ch % 2 == 0:
                            nc.vector.tensor_copy(
                                out=osb[:, ch * H:(ch + 1) * H],
                                in_=pso[:, off:off + H])
                        else:
                            nc.scalar.copy(osb[:, ch * H:(ch + 1) * H],
                                           pso[:, off:off + H])
                for ch in range(4):
                    nc.gpsimd.dma_start(
                        out_d[it * T + ch * 128:it * T + (ch + 1) * 128, :],
                        osb[:, ch * H:(ch + 1) * H])

    if not _SKIP_SPLIT:
        _split_waits(nc, mybir, _br)
    return nc


def _make_in_maps(host):
    (encA, encLe, encLo, tbls, tbls_hl, W2W1, W1bot, W2aug, b2c, S) = host
    dup = USE_DR and not DR_BROADCAST
    in_maps = []
    for cix in range(NCORES):
        s = slice(cix * NPC, (cix + 1) * NPC)

        def tilefmt(e, nn):
            a = e[:, :, s].reshape(e.shape[0], nn, NT, T).transpose(0, 2, 1, 3)
            if dup:
                a = np.stack([a, a], axis=3)  # [rows, NT, nn, 2, T]
                return np.ascontiguousarray(
                    a.reshape(e.shape[0], NT * nn * 2 * T))
            return np.ascontiguousarray(a.reshape(e.shape[0], NT * nn * T))

        in_maps.append({
            "encA": tilefmt(encA, NODES),
            "encLe": tilefmt(encLe, 8),
            "encLo": tilefmt(encLo, 8),
            "tbls": tbls, "tblshl": tbls_hl,
            "w2w1": W2W1, "w1bot": W1bot,
            "w2aug": W2aug, "b2c": b2c,
        })
    return in_maps


def kernel(**inputs) -> np.ndarray:
    from concourse.bass_utils import run_bass_kernel_spmd

    host = _build_host_tensors(**inputs)
    S = host[-1]
    if _CACHED.get("S") != S:
        _CACHED["nc"] = _build_program(S)
        _CACHED["S"] = S
    nc = _CACHED["nc"]
    in_maps = _make_in_maps(host)
    res = run_bass_kernel_spmd(nc, in_maps, list(range(NCORES)))
    out = np.concatenate([res.results[c]["out"] for c in range(NCORES)],
                         axis=0)
    return out.astype(np.float32)


# revision 7
# speedup vs baseline: 1.2248x; 1.0769x over previous
"""AbilityEncoder TRN2 kernel v4.

One-hot fp8 encodings x host-folded bf16 tables:
  psA = tblA42 @ encA (pairs of nodes per matmul), psB = tblB @ encL
  (even/odd child pairs per matmul), lvl1 = tblA@enc + W2W1@sh + W1bot@sl
  accumulated per half (node pairs), root analogous, output stage is a
  single W2aug-stationary matmul per tile writing H-major; the host
  transposes the final [H, NPC] result.
"""
import numpy as np
import ml_dtypes
import sys

sys.path.insert(0, "/opt/trn_rl_repo")

H = 96
NODES = 21
N = 32768
NCORES = 8
NPC = N // NCORES
T = 512
NT = NPC // T
RA = 42   # encA rows
RL = 26   # encL rows
F8 = ml_dtypes.float8_e4m3
BF = ml_dtypes.bfloat16

NFUSE = 2            # nodes per lvl2/lvl1 matmul (2 -> N=1024 streams)
# lvl2 pair-relu engine pattern: A=scalar(ACT), D=vector(DVE)
RELU_PAT = "ADAADADAADADAADA"  # 10 A, 6 D


def _build_host_tensors(trigger_ids, action_ids, target_ids, operand_ids,
                        trig_table, eff_table, targ_table, op_table,
                        W1, b1, W2, b2, Wl, bl):
    f64 = np.float64
    W1 = W1.astype(f64); W2 = W2.astype(f64); Wl = Wl.astype(f64)
    W1top, W1bot = W1[:H], W1[H:]
    W2W1 = W2 @ W1bot

    tblA = np.zeros((RA, H), f64)
    tblA[0:7] = trig_table.astype(f64) @ W1bot + b1.astype(f64)
    tblA[7:16] = eff_table.astype(f64) @ W1bot
    tblA[16:26] = targ_table.astype(f64) @ W1bot
    tblA[26:41] = op_table.astype(f64) @ W1top
    tblA[41] = b2.astype(f64) @ W1bot
    tblB = np.zeros((RL, H), f64)
    tblB[0:7] = trig_table.astype(f64) @ Wl + bl.astype(f64)
    tblB[7:16] = eff_table.astype(f64) @ Wl
    tblB[16:26] = targ_table.astype(f64) @ Wl

    tbls = np.zeros((128, H), f64)
    tbls[0:RA] = tblA
    tbls[64:64 + RL] = tblB
    tbls[96:96 + RL] = tblB

    W2aug = np.zeros((H + 1, H), f64)
    W2aug[:H] = W2
    W2aug[H] = b2.astype(f64)
    b2w1x4 = 4.0 * (b2.astype(f64) @ W1bot)

    t = trigger_ids.astype(np.int64); a = action_ids.astype(np.int64)
    g = target_ids.astype(np.int64); o = operand_ids.astype(np.int64)
    leaf = (o == 0)
    cols = np.arange(N)
    hasb2 = bool(np.any(b2 != 0))

    encA = np.zeros((RA, NODES, N), BF)
    encLe = np.zeros((RL, 8, N), BF)
    encLo = np.zeros((RL, 8, N), BF)
    for n in range(NODES):
        if n < 5:
            encA[:, n][t[:, n], cols] = 1.0
            encA[:, n][7 + a[:, n], cols] = 1.0
            encA[:, n][16 + g[:, n], cols] = 1.0
            encA[:, n][26 + o[:, n], cols] = 1.0
            if hasb2 and n >= 1:
                j = n - 1
                cnt = (~leaf[:, 5 + 4 * j:5 + 4 * j + 4]).sum(axis=1)
                encA[:, n][41, cols] = cnt.astype(np.float64)
        else:
            lf = leaf[:, n]
            nl = ~lf
            encA[:, n][t[nl, n], cols[nl]] = 1.0
            encA[:, n][7 + a[nl, n], cols[nl]] = 1.0
            encA[:, n][16 + g[nl, n], cols[nl]] = 1.0
            encA[:, n][26 + o[nl, n], cols[nl]] = 1.0
            p = n - 5
            dst = encLe if p % 2 == 0 else encLo
            k = p // 2
            dst[:, k][t[lf, n], cols[lf]] = 1.0
            dst[:, k][7 + a[lf, n], cols[lf]] = 1.0
            dst[:, k][16 + g[lf, n], cols[lf]] = 1.0

    return (encA, encLe, encLo, tbls.astype(BF),
            W2W1.astype(BF), W1bot.astype(BF), W2aug.astype(BF),
            b2w1x4.astype(np.float32).reshape(H, 1))


_CACHED = {}
_SKIP_SPLIT = False


def _patch_tile(tile, mybir, _br, ScopedClock):
    def _drain_and_barrier(self, tick_clock, wait_clock):
        nc_ = self.nc
        probe = nc_.sync.drain()
        wait_clock.add_sem_waits(probe.ins,
                                 ScopedClock({None: tick_clock.global_clock}))
        si = probe.ins.sync_info
        waits = list(si.on_wait) if si is not None else []
        if len(waits) > 1:
            si.on_wait = waits[:1]
            for w in waits[1:]:
                extra = nc_.sync.drain()
                extra.ins.sync_info = _br.SyncInfo(on_wait=[w], on_update=[])
        nc_.all_engine_barrier()
        popped = nc_._tile_sem_poison_stack.pop()
        assert popped is self._sem_poison
        nc_.clear_and_free_semaphores(list(self.sems.allocated().values()))
        nc_.all_engine_barrier()

    tile.TileContext._drain_and_barrier = _drain_and_barrier


def _split_waits(nc_, mybir, _br, max_waits=1):
    for f in nc_.m.functions:
        for bb in f.blocks:
            out = []
            for inst in bb.instructions:
                si = inst.sync_info
                if si is not None:
                    waits = list(si.on_wait)
                    if len(waits) > max_waits:
                        extra, keep = waits[:-max_waits], waits[-max_waits:]
                        for j, w in enumerate(extra):
                            ev = mybir.InstEventSemaphore(
                                name=f"{inst.name}-xw{j}")
                            ev.engine = inst.engine
                            ev.sync_info = _br.SyncInfo(
                                on_wait=[w], on_update=[])
                            out.append(ev)
                        si.on_wait = keep
                out.append(inst)
            try:
                bb.instructions = out
            except Exception:
                bb.instructions.clear()
                for i_ in out:
                    bb.instructions.append(i_)


def _build_program():
    import concourse.bass as bass
    import concourse.tile as tile
    import concourse.mybir as mybir
    from concourse.vector_clock import ScopedClock
    import bass_rust as _br

    _patch_tile(tile, mybir, _br, ScopedClock)

    dt = mybir.dt
    Relu = mybir.ActivationFunctionType.Relu
    ADD = mybir.AluOpType.add
    nc = bass.Bass(trn_type="TRN2", target_bir_lowering=False, debug=False)
    encA_d = nc.dram_tensor("encA", [RA, NT * NODES * T], dt.bfloat16,
                            kind="ExternalInput").ap()
    encLe_d = nc.dram_tensor("encLe", [RL, NT * 8 * T], dt.bfloat16,
                             kind="ExternalInput").ap()
    encLo_d = nc.dram_tensor("encLo", [RL, NT * 8 * T], dt.bfloat16,
                             kind="ExternalInput").ap()
    tbls_d = nc.dram_tensor("tbls", [128, H], dt.bfloat16,
                            kind="ExternalInput").ap()
    w2w1_d = nc.dram_tensor("w2w1", [H, H], dt.bfloat16,
                            kind="ExternalInput").ap()
    w1bot_d = nc.dram_tensor("w1bot", [H, H], dt.bfloat16,
                             kind="ExternalInput").ap()
    w2aug_d = nc.dram_tensor("w2aug", [H + 1, H], dt.bfloat16,
                             kind="ExternalInput").ap()
    b2c_d = nc.dram_tensor("b2c", [H, 1], dt.float32,
                           kind="ExternalInput").ap()
    out_d = nc.dram_tensor("out", [H, NPC], dt.float32,
                           kind="ExternalOutput").ap()

    with tile.TileContext(nc) as tc:
        with tc.tile_pool(name="const", bufs=1) as cpool, \
             tc.tile_pool(name="enc", bufs=3) as epool, \
             tc.tile_pool(name="hbuf", bufs=3) as hpool, \
             tc.tile_pool(name="ps", bufs=4, space="PSUM") as pspool:

            tbls_s = cpool.tile([128, H], dt.bfloat16)
            nc.sync.dma_start(tbls_s[:], tbls_d[:])
            w2w1_s = cpool.tile([H, H], dt.bfloat16)
            nc.sync.dma_start(w2w1_s[:], w2w1_d[:])
            w1bot_s = cpool.tile([H, H], dt.bfloat16)
            nc.sync.dma_start(w1bot_s[:], w1bot_d[:])
            w2aug_s = cpool.tile([H + 1, H], dt.bfloat16)
            nc.sync.dma_start(w2aug_s[:], w2aug_d[:])
            b2c_s = cpool.tile([H, 1], dt.float32)
            nc.sync.dma_start(b2c_s[:], b2c_d[:])

            tbA = tbls_s[0:RA, :]
            tbBe = tbls_s[64:64 + RL, :]
            tbBo = tbls_s[96:96 + RL, :]

            for it in range(NT):
                enc = epool.tile([128, NODES * T], dt.bfloat16, tag="enc")
                nc.sync.dma_start(
                    enc[0:RA, :],
                    encA_d[:, it * NODES * T:(it + 1) * NODES * T])
                nc.sync.dma_start(
                    enc[64:64 + RL, 0:8 * T],
                    encLe_d[:, it * 8 * T:(it + 1) * 8 * T])
                nc.sync.dma_start(
                    enc[96:96 + RL, 0:8 * T],
                    encLo_d[:, it * 8 * T:(it + 1) * 8 * T])

                def eA(n):
                    return enc[0:RA, n * T:(n + 1) * T]

                def eL(p):
                    k = p // 2
                    row0 = 64 if p % 2 == 0 else 96
                    return enc[row0:row0 + RL, k * T:(k + 1) * T]

                hA = []
                hB = []
                nrelu = 0

                def evac(dst, src):
                    nonlocal nrelu
                    if RELU_PAT[nrelu] == "A":
                        nc.scalar.activation(dst, src, Relu)
                    else:
                        nc.vector.tensor_scalar_max(dst, src, 0.0)
                    nrelu += 1

                for j in range(4):
                    hA_j = hpool.tile([H, 4, T], dt.bfloat16, tag="hA")
                    hB_j = hpool.tile([H, 4, T], dt.bfloat16, tag="hB")
                    hA.append(hA_j)
                    hB.append(hB_j)
                    for k in range(2):
                        psA = pspool.tile([128, 2 * T], dt.float32, tag="ps")
                        psB = pspool.tile([128, 2 * T], dt.float32, tag="ps")
                        for c in (2 * k, 2 * k + 1):
                            n = 5 + 4 * j + c
                            p = n - 5
                            o_ = (c - 2 * k) * T
                            nc.tensor.matmul(psA[0:H, o_:o_ + T], tbA, eA(n),
                                             start=True, stop=True)
                            if p % 2 == 0:
                                nc.tensor.matmul(psB[0:H, o_:o_ + T], tbBe,
                                                 eL(p), start=True, stop=True,
                                                 tile_position=(64, 0))
                            else:
                                nc.tensor.matmul(psB[0:H, o_:o_ + T], tbBo,
                                                 eL(p), start=True, stop=True,
                                                 tile_position=(96, 0))
                        evac(hA_j[:, 2 * k:2 * k + 2, :], psA[0:H, :])
                        evac(hB_j[:, 2 * k:2 * k + 2, :], psB[0:H, :])

                # sibling sums
                sh = []
                sl = []
                for j in range(4):
                    sh_j = hpool.tile([H, T], dt.bfloat16, tag="sh")
                    sl_j = hpool.tile([H, T], dt.bfloat16, tag="sl")
                    sh.append(sh_j)
                    sl.append(sl_j)
                    tmpA = hpool.tile([H, 2, T], dt.bfloat16, tag="tmpA")
                    nc.vector.tensor_tensor(out=tmpA[:], in0=hA[j][:, 0:2, :],
                                            in1=hA[j][:, 2:4, :], op=ADD)
                    nc.vector.tensor_tensor(out=sh_j[:], in0=tmpA[:, 0, :],
                                            in1=tmpA[:, 1, :], op=ADD)
                    tmpB = hpool.tile([H, 2, T], dt.bfloat16, tag="tmpB")
                    nc.vector.tensor_tensor(out=tmpB[:], in0=hB[j][:, 0:2, :],
                                            in1=hB[j][:, 2:4, :], op=ADD)
                    nc.vector.tensor_tensor(out=sl_j[:], in0=tmpB[:, 0, :],
                                            in1=tmpB[:, 1, :], op=ADD)

                # level 1 + root prim: weight-grouped matmul chains
                h1 = hpool.tile([H, 4, T], dt.bfloat16, tag="h1")
                ps1a = pspool.tile([128, 2 * T], dt.float32, tag="ps")
                ps1b = pspool.tile([128, 2 * T], dt.float32, tag="ps")
                ps0 = pspool.tile([128, 2 * T], dt.float32, tag="ps")
                ps1 = (ps1a, ps1b)

                def dst1(j):
                    return ps1[j // 2][0:H, (j % 2) * T:(j % 2 + 1) * T]

                nc.tensor.matmul(ps0[0:H, 0:T], tbA, eA(0),
                                 start=True, stop=False)
                for j in range(4):
                    nc.tensor.matmul(dst1(j), tbA, eA(1 + j),
                                     start=True, stop=False)
                for j in range(4):
                    nc.tensor.matmul(dst1(j), w2w1_s[:], sh[j][:],
                                     start=False, stop=False)
                for j in range(4):
                    nc.tensor.matmul(dst1(j), w1bot_s[:], sl[j][:],
                                     start=False, stop=True)
                for half in range(2):
                    nc.scalar.activation(h1[:, 2 * half:2 * half + 2, :],
                                         ps1[half][0:H, :], Relu)

                # root
                s0 = hpool.tile([H, T], dt.bfloat16, tag="s0")
                tmp0 = hpool.tile([H, 2, T], dt.bfloat16, tag="tmp0")
                nc.vector.tensor_tensor(out=tmp0[:], in0=h1[:, 0:2, :],
                                        in1=h1[:, 2:4, :], op=ADD)
                nc.vector.tensor_tensor(out=s0[:], in0=tmp0[:, 0, :],
                                        in1=tmp0[:, 1, :], op=ADD)
                nc.tensor.matmul(ps0[0:H, 0:T], w2w1_s[:], s0[:],
                                 start=False, stop=True)
                h0 = hpool.tile([H + 1, T], dt.bfloat16, tag="h0")
                nc.scalar.activation(h0[0:H, :], ps0[0:H, 0:T], Relu,
                                     bias=b2c_s[:])
                nc.vector.memset(h0[H:H + 1, :], 1.0)

                # out.T = W2aug.T @ h0  (H-major; host transposes at the end)
                nc.tensor.matmul(ps0[0:H, T:2 * T], w2aug_s[:], h0[:],
                                 start=True, stop=True)
                osb = hpool.tile([H, T], dt.float32, tag="osb")
                if it % 2 == 0:
                    nc.scalar.copy(osb[:], ps0[0:H, T:2 * T])
                else:
                    nc.vector.tensor_copy(out=osb[:], in_=ps0[0:H, T:2 * T])
                nc.gpsimd.dma_start(out_d[:, it * T:(it + 1) * T], osb[:])

    if not _SKIP_SPLIT:
        _split_waits(nc, mybir, _br)
    return nc


def _make_in_maps(host):
    (encA, encLe, encLo, tbls, W2W1, W1bot, W2aug, b2c) = host
    in_maps = []
    for cix in range(NCORES):
        s = slice(cix * NPC, (cix + 1) * NPC)

        def tilefmt(e, nn):
            a = e[:, :, s].reshape(e.shape[0], nn, NT, T).transpose(0, 2, 1, 3)
            return np.ascontiguousarray(a.reshape(e.shape[0], NT * nn * T))

        in_maps.append({
            "encA": tilefmt(encA, NODES),
            "encLe": tilefmt(encLe, 8),
            "encLo": tilefmt(encLo, 8),
            "tbls": tbls, "w2w1": W2W1, "w1bot": W1bot,
            "w2aug": W2aug, "b2c": b2c,
        })
    return in_maps


def kernel(**inputs) -> np.ndarray:
    from concourse.bass_utils import run_bass_kernel_spmd

    host = _build_host_tensors(**inputs)
    if "nc" not in _CACHED:
        _CACHED["nc"] = _build_program()
    nc = _CACHED["nc"]
    in_maps = _make_in_maps(host)
    res = run_bass_kernel_spmd(nc, in_maps, list(range(NCORES)))
    out = np.concatenate(
        [res.results[c]["out"].T for c in range(NCORES)], axis=0)
    return np.ascontiguousarray(out).astype(np.float32)
